# revision 1
# baseline (speedup 1.0000x reference)
"""Trainium2 Bass kernel for nn_DeepRecursiveNetwork.

Math (reference): 30 outer steps; each step, per block n (0..9):
    inp  = h[n] + block_in[n]           (block_in = x_emb for n=0 else h[n-1] from prev step)
    inner equilibrium, 5 iters from h'=0:
        h' = 0.5 h' + 0.5 tanh(h' @ W[n].T + b[n] + inp)
    h[n] = 0.5 h[n] + 0.5 h'
Output: h[9] @ head_W.T + head_b.

Device formulation (per core, 8-way data parallel over batch, B_local=128):
  - All recurrent tensors live TRANSPOSED in SBUF as [128, 8*128] tiles laid
    out (d_lo, (d_hi, b)) so matmuls (out = lhsT.T @ rhs, contraction on the
    partition dim) need no transposes anywhere.
  - Inner state substitution u = 2*h' with pre-halved weights Wt = W.T/2:
        u_{j+1} = 0.5*u_j + tanh(Wt.T-matmul(u_j) + c + b[n])
    (one fused scalar_tensor_tensor per tile), u_1 = tanh(c + b[n]),
    h'_5 = u_5/2, outer update v[n] = 0.5 v[n] + 0.25 u_5.
  - Blocks processed in reverse order per step so block n reads the
    previous-step value of v[n-1] with pure in-place updates.
  - Matmuls in fp16 (full PE rate); two complementary fp16 roundings of the
    weights are used on alternating outer steps so the correlated rounding
    bias cancels (measured ~0.5% max err vs 1.0% with plain fp16 rounding).
    All elementwise math is fp32; PSUM accumulation is fp32.
  - Weights (2 x 20 MB fp16) stream from HBM per (block, step), double
    buffered; 2 MB per block-step against ~20 us of PE work.
  - Blocks are processed as software-pipelined pairs (n, n-1): within a step
    they are data-independent, so their matmul iterations interleave on the
    PE and each block's psum->add->tanh->axpy chain hides under the other
    block's 64-matmul group instead of stalling the PE.
"""

import numpy as np

import concourse.bass as bass
import concourse.bacc as bacc
import concourse.mybir as mybir
from concourse.bass_utils import run_bass_kernel_spmd
from concourse.tile import TileContext

F32 = mybir.dt.float32
F16 = mybir.dt.float16

B, DIN, H, DOUT, NB = 1024, 512, 1024, 512, 10
NCORES = 8
BL = B // NCORES  # 128 batch per core
KH = H // 128     # 8 k/m tiles over H
KD = DIN // 128   # 4 k tiles over DIN
KO = DOUT // 128  # 4 m tiles over DOUT
INNER = 5
Tanh = mybir.ActivationFunctionType.Tanh
Ident = mybir.ActivationFunctionType.Identity
MULT = mybir.AluOpType.mult
ADD = mybir.AluOpType.add


def build_nc(steps: int):
    nc = bacc.Bacc(None, target_bir_lowering=False)
    xT = nc.dram_tensor("xT", [128, KD * BL], F32, kind="ExternalInput")
    embWT = nc.dram_tensor("embWT", [128, KD * H], F32, kind="ExternalInput")
    embB = nc.dram_tensor("embB", [128, KH], F32, kind="ExternalInput")
    Wab = nc.dram_tensor("Wab", [2, NB, 128, KH * H], F16, kind="ExternalInput")
    bT = nc.dram_tensor("bT", [128, NB * KH], F32, kind="ExternalInput")
    headWT = nc.dram_tensor("headWT", [128, KH * DOUT], F32, kind="ExternalInput")
    headB = nc.dram_tensor("headB", [128, KO], F32, kind="ExternalInput")
    outT = nc.dram_tensor("outT", [128, KO * BL], F32, kind="ExternalOutput")

    with TileContext(nc) as tc:
        with (
            tc.tile_pool(name="const", bufs=1) as cpool,
            tc.tile_pool(name="state", bufs=1) as spool,
            tc.tile_pool(name="wts", bufs=2) as wpool,
            tc.tile_pool(name="work", bufs=2) as kpool,
            tc.tile_pool(name="small", bufs=8) as mpool,
            tc.tile_pool(name="psum", bufs=1, space="PSUM") as ppool,
        ):
            # ---- constants ----
            xT_sb = cpool.tile([128, KD * BL], F32, tag="xt", bufs=1)
            embWT_sb = cpool.tile([128, KD * H], F32, tag="embwt", bufs=1)
            embB_sb = cpool.tile([128, KH], F32, tag="embb", bufs=1)
            bT_sb = cpool.tile([128, NB * KH], F32, tag="bt", bufs=1)
            headWT_sb = cpool.tile([128, KH * DOUT], F32, tag="hwt", bufs=1)
            headB_sb = cpool.tile([128, KO], F32, tag="hb", bufs=1)
            nc.gpsimd.dma_start(xT_sb[:], xT[:])
            nc.gpsimd.dma_start(embWT_sb[:], embWT[:])
            nc.gpsimd.dma_start(embB_sb[:], embB[:])
            nc.gpsimd.dma_start(bT_sb[:], bT[:])
            nc.gpsimd.dma_start(headWT_sb[:], headWT[:])
            nc.gpsimd.dma_start(headB_sb[:], headB[:])
            # Stage every constant through a DVE copy: downstream consumers
            # then depend on a single (DVE) semaphore. Self-loading fp32
            # matmuls only have ONE sync-wait slot in their LW struct, so
            # they cannot wait on two DMA queues directly.
            xT2 = cpool.tile([128, KD * BL], F32, tag="xt2", bufs=1)
            embWT2 = cpool.tile([128, KD * H], F32, tag="embwt2", bufs=1)
            embB2 = cpool.tile([128, KH], F32, tag="embb2", bufs=1)
            bT2 = cpool.tile([128, NB * KH], F32, tag="bt2", bufs=1)
            headWT2 = cpool.tile([128, KH * DOUT], F32, tag="hwt2", bufs=1)
            headB2 = cpool.tile([128, KO], F32, tag="hb2", bufs=1)
            for dst, srcv in ((xT2, xT_sb), (embWT2, embWT_sb), (embB2, embB_sb),
                              (bT2, bT_sb), (headWT2, headWT_sb), (headB2, headB_sb)):
                nc.vector.tensor_copy(dst[:], srcv[:])
            xT_sb, embWT_sb, embB_sb, bT_sb, headWT_sb, headB_sb = (
                xT2, embWT2, embB2, bT2, headWT2, headB2)

            # ---- persistent state (transposed layout) ----
            v = [spool.tile([128, H], F32, tag=f"v{n}", bufs=1, name=f"v{n}") for n in range(NB)]
            xemb = spool.tile([128, H], F32, tag="xemb", bufs=1)

            # per-m-tile PSUM banks (each [128,128] fp32 tile pads to one bank)
            def psm(m):
                return ppool.tile([128, 128], F32, tag=f"ps{m}", bufs=1, name=f"ps{m}")

            for n in range(NB):
                nc.vector.memset(v[n][:], 0.0)

            # ---- embed: xemb = (x @ embed_W.T + embed_b)^T ----
            for m in range(KH):
                pe = psm(m)
                for k in range(KD):
                    nc.tensor.matmul(
                        pe[:],
                        embWT_sb[:, k * H + m * 128 : k * H + (m + 1) * 128],
                        xT_sb[:, k * BL : (k + 1) * BL],
                        start=(k == 0),
                        stop=(k == KD - 1),
                    )
                nc.scalar.activation(
                    xemb[:, m * 128 : (m + 1) * 128], pe[:], Ident,
                    bias=embB_sb[:, m : m + 1], scale=1.0,
                )

            # ---- main recurrence ----
            # Blocks n and n-1 within a step are mutually independent (each
            # reads only previous-step state), so process them as a software-
            # pipelined pair: their matmul iterations interleave on the PE and
            # each block's psum->add->tanh->axpy chain hides under the other
            # block's 64-matmul group instead of stalling the PE. PSUM layout
            # (one bank per m-tile) is unchanged from the validated baseline.
            for step in range(steps):
                par = step % 2
                for pn in range(NB - 1, 0, -2):
                    pair = (pn, pn - 1)
                    wsets, cs, vhs, us = {}, {}, {}, {}
                    for n in pair:
                        w = []
                        for k in range(KH):
                            wk = wpool.tile([128, H], F16, tag=f"w{k}", bufs=2, name=f"w{k}")
                            nc.sync.dma_start(
                                wk[:], Wab[par, n, :, k * H : (k + 1) * H]
                            )
                            w.append(wk)
                        wsets[n] = w

                        binT = xemb if n == 0 else v[n - 1]
                        c = kpool.tile([128, H], F32, tag="c", bufs=3, name="c")
                        nc.vector.tensor_add(c[:], v[n][:], binT[:])
                        vh = kpool.tile([128, H], F32, tag="vh", bufs=3, name="vh")
                        nc.vector.tensor_scalar_mul(vh[:], v[n][:], 0.5)
                        cs[n], vhs[n] = c, vh

                        # u1 = tanh(c + b[n])  (inner iter 0; state is zero)
                        u = kpool.tile([128, H], F16, tag="u", bufs=4, name="u")
                        for m in range(KH):
                            nc.scalar.activation(
                                u[:, m * 128 : (m + 1) * 128],
                                c[:, m * 128 : (m + 1) * 128],
                                Tanh, bias=bT_sb[:, n * KH + m : n * KH + m + 1],
                                scale=1.0,
                            )
                        us[n] = u

                    for j in range(1, INNER):
                        last = j == INNER - 1
                        for n in pair:
                            u, c, vh, w = us[n], cs[n], vhs[n], wsets[n]
                            un = None if last else kpool.tile(
                                [128, H], F16, tag="u", bufs=4, name="un"
                            )
                            for m in range(KH):
                                mc = slice(m * 128, (m + 1) * 128)
                                ps = psm(m)
                                for k in range(KH):
                                    nc.tensor.matmul(
                                        ps[:],
                                        w[k][:, m * 128 : (m + 1) * 128],
                                        u[:, k * 128 : (k + 1) * 128],
                                        start=(k == 0),
                                        stop=(k == KH - 1),
                                    )
                                s = mpool.tile([128, 128], F32, tag="s", bufs=8)
                                nc.vector.tensor_add(s[:], ps[:], c[:, mc])
                                t = mpool.tile([128, 128], F32, tag="t", bufs=8)
                                nc.scalar.activation(
                                    t[:], s[:], Tanh,
                                    bias=bT_sb[:, n * KH + m : n * KH + m + 1],
                                    scale=1.0,
                                )
                                if not last:
                                    nc.vector.scalar_tensor_tensor(
                                        un[:, mc], u[:, mc], 0.5, t[:], MULT, ADD
                                    )
                                else:
                                    u5 = mpool.tile([128, 128], F32, tag="u5", bufs=8)
                                    nc.vector.scalar_tensor_tensor(
                                        u5[:], u[:, mc], 0.5, t[:], MULT, ADD
                                    )
                                    nc.vector.scalar_tensor_tensor(
                                        v[n][:, mc], u5[:], 0.25, vh[:, mc], MULT, ADD
                                    )
                            if not last:
                                us[n] = un

            # ---- head: out^T = head_W @ v[9]^T + head_b ----
            outsb = kpool.tile([128, KO * BL], F32, tag="outsb", bufs=1)
            for m in range(KO):
                ph = psm(m)
                for k in range(KH):
                    nc.tensor.matmul(
                        ph[:],
                        headWT_sb[:, k * DOUT + m * 128 : k * DOUT + (m + 1) * 128],
                        v[NB - 1][:, k * 128 : (k + 1) * 128],
                        start=(k == 0),
                        stop=(k == KH - 1),
                    )
                nc.scalar.activation(
                    outsb[:, m * BL : (m + 1) * BL], ph[:], Ident,
                    bias=headB_sb[:, m : m + 1], scale=1.0,
                )
            nc.sync.dma_start(outT[:], outsb[:])
    nc.compile()
    return nc


def _tile_k(a):
    """[K, M] -> [128, (K//128)*M] laid out (k_lo, k_hi, m)."""
    K, M = a.shape
    return np.ascontiguousarray(
        a.reshape(K // 128, 128, M).transpose(1, 0, 2).reshape(128, (K // 128) * M)
    )


def kernel(**inputs) -> np.ndarray:
    x = np.asarray(inputs["x"], np.float32)
    embed_W = np.asarray(inputs["embed_W"], np.float32)
    embed_b = np.asarray(inputs["embed_b"], np.float32)
    block_W = np.asarray(inputs["block_W"], np.float32)
    block_b = np.asarray(inputs["block_b"], np.float32)
    head_W = np.asarray(inputs["head_W"], np.float32)
    head_b = np.asarray(inputs["head_b"], np.float32)
    steps = int(np.asarray(inputs["steps"]))

    embWT = _tile_k(embed_W.T)
    headWT = _tile_k(head_W.T)
    Wt = block_W.transpose(0, 2, 1) * np.float32(0.5)  # [NB, K=h_in, M=d_out]
    Wa = Wt.astype(np.float16)
    Wb = (2.0 * Wt - Wa.astype(np.float32)).astype(np.float16)
    Wab = np.stack(
        [
            np.stack([_tile_k(Wa[n]) for n in range(NB)]),
            np.stack([_tile_k(Wb[n]) for n in range(NB)]),
        ]
    )  # [2, NB, 128, 8*1024] f16
    embB = np.ascontiguousarray(embed_b.reshape(KH, 128).T)
    bT = np.ascontiguousarray(
        block_b.reshape(NB, KH, 128).transpose(2, 0, 1).reshape(128, NB * KH)
    )
    headB = np.ascontiguousarray(head_b.reshape(KO, 128).T)

    in_maps = []
    for ci in range(NCORES):
        xT = _tile_k(np.ascontiguousarray(x[ci * BL : (ci + 1) * BL].T))
        in_maps.append(
            dict(xT=xT, embWT=embWT, embB=embB, Wab=Wab, bT=bT,
                 headWT=headWT, headB=headB)
        )

    nc = build_nc(steps)
    res = run_bass_kernel_spmd(nc, in_maps, core_ids=list(range(NCORES)))

    out = np.empty((B, DOUT), np.float32)
    for ci in range(NCORES):
        oT = res.results[ci]["outT"]  # [128, (do_hi=4, b=128)] = out^T tiled
        out[ci * BL : (ci + 1) * BL] = (
            oT.reshape(128, KO, BL).transpose(2, 1, 0).reshape(BL, DOUT)
        )
    return out



# revision 18
# speedup vs baseline: 1.8747x; 1.8747x over previous
"""Trainium2 Bass kernel for nn_DeepRecursiveNetwork.

Math (reference): 30 outer steps; each step, per block n (0..9):
    inp  = h[n] + block_in[n]           (block_in = x_emb for n=0 else h[n-1] from prev step)
    inner equilibrium, 5 iters from h'=0:
        h' = 0.5 h' + 0.5 tanh(h' @ W[n].T + b[n] + inp)
    h[n] = 0.5 h[n] + 0.5 h'
Output: h[9] @ head_W.T + head_b.

Device formulation (per core, 8-way data parallel over batch, B_local=128):
  - All recurrent tensors live TRANSPOSED in SBUF as [128, 8*128] tiles laid
    out (d_lo, (d_hi, b)) so matmuls (out = lhsT.T @ rhs, contraction on the
    partition dim) need no transposes anywhere.
  - Inner state substitution u = 2*h' with pre-halved weights Wt = W.T/2:
        u_{k+1} = 0.5*u_k + tanh(Wt.T-matmul(u_k) + c + b[n])
    u_1 = tanh(c + b[n]), outer update v[n] = 0.5 v[n] + 0.25 u_5.
  - Matmuls in fp16; two complementary fp16 roundings of the weights are used
    on alternating outer steps so the correlated rounding bias cancels.
    All elementwise math is fp32 internally; PSUM accumulation is fp32.
  - Wavefront skips: with zero block biases, block n's state is exactly zero
    until step n (zeros propagate: tanh(0)=0), and block n's updates after
    step steps-NB+n cannot reach the head output (shortest path to block 9
    takes 9-n steps).  Both skips are exact; 300 -> 210 block-step tasks.
  - Elementwise work uses wide instructions (two [128,512] halves or full
    [128,1024]): per inner round one DVE add (psum+cb), one ACT tanh, one
    DVE axpy (4x-mode ts_mul + 2x-mode f16 tensor_tensor); the cb add runs
    on the otherwise idle gpsimd engine.  State v and activations are fp16;
    pre-activations stay fp32.
  - Tasks (step, block) run as a 5-stage skewed software pipeline
    [setup+round1, round2, round3, round4, v-update] with one new task per
    slot, so the PE sees four consecutive 64-MM batches from four different
    tasks and never drains (any PE idle would also drop the cost-model
    p-state from 2.4 to 1.2 GHz for 3us).  Each round's axpy is emitted one
    slot after its tanh so engine queues never head-of-line block.
  - Per-task start slots respect cross-task RAW order: a task reading v[n]
    is emitted only after the previous writer's final stage, which inserts
    stall slots at the sparse wavefront head/tail (without this the skewed
    emission reads stale state - program order defines the dataflow).
  - PSUM = eight 1-bank [128,512] tiles = four (A,B) region pairs rotating
    across in-flight tasks.
  - Weights (20 MB fp16 per rounding set) stream from HBM per (block, step),
    one [128, 8192] DMA each, quadruple buffered.  Head constants load last.
"""

import numpy as np

import concourse.bacc as bacc
import concourse.mybir as mybir
from concourse.bass_utils import run_bass_kernel_spmd
from concourse.tile import TileContext

F32 = mybir.dt.float32
F16 = mybir.dt.float16

B, DIN, H, DOUT, NB = 1024, 512, 1024, 512, 10
NCORES = 8
BL = B // NCORES  # 128 batch per core
KH = H // 128     # 8 k/m tiles over H
KD = DIN // 128   # 4 k tiles over DIN
KO = DOUT // 128  # 4 m tiles over DOUT
INNER = 5
Tanh = mybir.ActivationFunctionType.Tanh
Ident = mybir.ActivationFunctionType.Identity
MULT = mybir.AluOpType.mult
ADD = mybir.AluOpType.add


def active_blocks(step: int, steps: int, skip_fwd: bool):
    """Blocks whose update at `step` is needed (descending order)."""
    ns = []
    for n in range(NB - 1, -1, -1):
        if skip_fwd and n > step:
            continue  # state still exactly zero
        if step > steps - NB + n:
            continue  # cannot influence block NB-1 by the last step
        ns.append(n)
    return ns


def build_nc(steps: int, zero_bias: bool = True, debug: bool = False, skips: bool = True):
    nc = bacc.Bacc(None, target_bir_lowering=False)
    xT = nc.dram_tensor("xT", [128, KD * BL], F32, kind="ExternalInput")
    embWT = nc.dram_tensor("embWT", [128, KD * H], F32, kind="ExternalInput")
    embB = nc.dram_tensor("embB", [128, KH], F32, kind="ExternalInput")
    Wab = nc.dram_tensor("Wab", [2, NB, 128, KH * H], F16, kind="ExternalInput")
    bF = nc.dram_tensor("bF", [128, NB * H], F16, kind="ExternalInput")
    headWT = nc.dram_tensor("headWT", [128, KH * DOUT], F16, kind="ExternalInput")
    headB = nc.dram_tensor("headB", [128, KO], F32, kind="ExternalInput")
    outT = nc.dram_tensor("outT", [128, KO * BL], F32, kind="ExternalOutput")
    dbg = None
    if debug:
        dbg = nc.dram_tensor("dbg", [128, (NB + 1) * H], F32, kind="ExternalOutput")

    with TileContext(nc) as tc:
        with (
            tc.tile_pool(name="const", bufs=1) as cpool,
            tc.tile_pool(name="state", bufs=1) as spool,
            tc.tile_pool(name="wts", bufs=3) as wpool,
            tc.tile_pool(name="work", bufs=2) as kpool,
            tc.tile_pool(name="psum", bufs=1, space="PSUM") as ppool,
        ):
            # ---- constants ----
            xT_sb = cpool.tile([128, KD * BL], F32, tag="xt", bufs=1)
            embWT_sb = cpool.tile([128, KD * H], F32, tag="embwt", bufs=1)
            embB_sb = cpool.tile([128, KH], F32, tag="embb", bufs=1)
            headWT_sb = cpool.tile([128, KH * DOUT], F16, tag="hwt", bufs=1)
            headB_sb = cpool.tile([128, KO], F32, tag="hb", bufs=1)
            nc.gpsimd.dma_start(xT_sb[:], xT[:])
            nc.gpsimd.dma_start(embWT_sb[:], embWT[:])
            nc.gpsimd.dma_start(embB_sb[:], embB[:])
            bF_sb = None
            if not zero_bias:
                bF_sb = cpool.tile([128, NB * H], F16, tag="bf", bufs=1)
                nc.gpsimd.dma_start(bF_sb[:], bF[:])
            # Stage every constant through a DVE copy: downstream consumers
            # then depend on a single (DVE) semaphore.  Self-loading fp32
            # matmuls only have ONE sync-wait slot in their LW struct, so
            # they cannot wait on two DMA queues directly.
            xT2 = cpool.tile([128, KD * BL], F32, tag="xt2", bufs=1)
            embWT2 = cpool.tile([128, KD * H], F32, tag="embwt2", bufs=1)
            embB2 = cpool.tile([128, KH], F32, tag="embb2", bufs=1)
            headWT2 = cpool.tile([128, KH * DOUT], F16, tag="hwt2", bufs=1)
            headB2 = cpool.tile([128, KO], F32, tag="hb2", bufs=1)
            stage = [(xT2, xT_sb), (embWT2, embWT_sb), (embB2, embB_sb)]
            for dst, srcv in stage:
                nc.vector.tensor_copy(dst[:], srcv[:])
            xT_sb, embWT_sb, embB_sb = xT2, embWT2, embB2

            # ---- persistent state (transposed layout, fp16) ----
            v = [spool.tile([128, H], F16, tag=f"v{n}", bufs=1, name=f"v{n}")
                 for n in range(NB)]
            xemb = spool.tile([128, H], F16, tag="xemb", bufs=1)
            for n in range(NB):
                nc.vector.memset(v[n][:], 0.0)

            # eight 1-bank PSUM halves -> four (A, B) region pairs
            psh = [ppool.tile([128, 512], F32, tag=f"ps{i}", bufs=1, name=f"ps{i}")
                   for i in range(8)]
            psr = [(psh[2 * i], psh[2 * i + 1]) for i in range(4)]

            # ---- embed: xemb = (x @ embed_W.T + embed_b)^T  (fp32 matmuls) ----
            for m in range(KH):
                pe = psr[0][0] if m < 4 else psr[0][1]
                mc = slice((m % 4) * 128, (m % 4 + 1) * 128)
                for k in range(KD):
                    nc.tensor.matmul(
                        pe[:, mc],
                        embWT_sb[:, k * H + m * 128 : k * H + (m + 1) * 128],
                        xT_sb[:, k * BL : (k + 1) * BL],
                        start=(k == 0),
                        stop=(k == KD - 1),
                    )
                nc.scalar.activation(
                    xemb[:, m * 128 : (m + 1) * 128], pe[:, mc], Ident,
                    bias=embB_sb[:, m : m + 1], scale=1.0,
                )

            if debug:
                dx = kpool.tile([128, H], F32, tag="dx", bufs=1)
                nc.vector.tensor_copy(dx[:], xemb[:])
                nc.sync.dma_start(dbg[:, NB * H : (NB + 1) * H], dx[:])

            # ---- main recurrence (skewed software pipeline) ----
            tasks = []
            for step in range(steps):
                for n in (active_blocks(step, steps, zero_bias) if skips else range(NB - 1, -1, -1)):
                    tasks.append((step, n))

            def emit_mms(st):
                """64-MM batch for the current round (moving operand st["u"])."""
                (psA, psB), wt, u = st["ps"], st["wt"], st["u"]
                for m in range(KH):
                    ph = psA if m < 4 else psB
                    mc = slice((m % 4) * 128, (m % 4 + 1) * 128)
                    for kk in range(KH):
                        nc.tensor.matmul(
                            ph[:, mc],
                            wt[:, kk * H + m * 128 : kk * H + (m + 1) * 128],
                            u[:, kk * 128 : (kk + 1) * 128],
                            start=(kk == 0),
                            stop=(kk == KH - 1),
                        )

            def emit_s_t(st):
                """s = ps + cb; t = tanh(s), split into the two PSUM-bank
                halves so the chain for the first half starts as soon as the
                m=0..3 matmul groups finish (~1.7us before the full batch)."""
                ts_ = []
                for h_ in range(2):
                    hc = slice(h_ * 512, (h_ + 1) * 512)
                    s = kpool.tile([128, 512], F32, tag=f"s{h_}", bufs=(3 if not zero_bias else 4), name="s")
                    nc.vector.tensor_add(s[:], st["ps"][h_][:], st["cb"][:, hc])
                    t = kpool.tile([128, 512], F16, tag=f"t{h_}", bufs=(4 if not zero_bias else 6), name="t")
                    nc.scalar.activation(t[:], s[:], Tanh, bias=0.0, scale=1.0)
                    ts_.append(t)
                st["t"] = ts_

            def emit_axpy(st):
                """u <- 0.5 u + t (previous round's t, already computed)."""
                u, (tA, tB) = st["u"], st["t"]
                # ts_mul runs in the DVE 4x mode, tt-add in 2x: 921ns total
                # vs 1127ns for the fused scalar_tensor_tensor.
                uh = kpool.tile([128, H], F16, tag="uh", bufs=3, name="uh")
                nc.vector.tensor_scalar_mul(uh[:], u[:], 0.5)
                un = kpool.tile([128, H], F16, tag="u", bufs=(6 if not zero_bias else 7), name="un")
                nc.vector.tensor_add(un[:, 0:512], uh[:, 0:512], tA[:])
                nc.vector.tensor_add(un[:, 512:1024], uh[:, 512:1024], tB[:])
                st["u"] = un

            def emit_phase1(st, j):
                """Early work: weight DMA, cb, u1 (j=0); axpy (j>=1); v-update
                (j=4).  All inputs were produced in earlier slots, so these
                never stall the engine queues."""
                step, n = st["task"]
                if j == 0:
                    par = step % 2
                    wt = wpool.tile([128, KH * H], F16, tag="w", bufs=(3 if (debug or not zero_bias) else 4), name="w")
                    nc.sync.dma_start(wt[:], Wab[par, n, :, :])
                    st["wt"] = wt
                    binT = xemb if n == 0 else v[n - 1]
                    cb = kpool.tile([128, H], F16, tag="cb", bufs=(4 if not zero_bias else 5), name="cb")
                    # cb on the (otherwise idle) gpsimd engine
                    nc.gpsimd.tensor_tensor(cb[:], v[n][:], binT[:], ADD)
                    if not zero_bias:
                        cb2 = kpool.tile([128, H], F16, tag="cb2", bufs=3, name="cb2")
                        nc.gpsimd.tensor_tensor(
                            cb2[:], cb[:], bF_sb[:, n * H : (n + 1) * H], ADD
                        )
                        cb = cb2
                    st["cb"] = cb
                    # u1 = tanh(cb)   (inner iter 0; state is zero)
                    u = kpool.tile([128, H], F16, tag="u", bufs=(6 if not zero_bias else 7), name="u")
                    nc.scalar.activation(u[:], cb[:], Tanh, bias=0.0, scale=1.0)
                    st["u"] = u
                else:
                    emit_axpy(st)
                    if j == 4:
                        # v[n] = 0.5 v[n] + 0.25 u5
                        vh = kpool.tile([128, H], F16, tag="vh", bufs=2, name="vh")
                        nc.vector.tensor_scalar_mul(vh[:], v[n][:], 0.5)
                        uq = kpool.tile([128, H], F16, tag="uq", bufs=2, name="uq")
                        nc.vector.tensor_scalar_mul(uq[:], st["u"][:], 0.25)
                        nc.vector.tensor_add(v[n][:], vh[:], uq[:])

            NSTAGE = 5
            T = len(tasks)
            # Per-task start slots.  A task READS v[n] and v[n-1] at its
            # first stage (S0) and WRITES v[n] at its last (S4).  Emission
            # (= program) order must put every read of v[k] after the
            # previous writer's S4, otherwise the reader picks up the stale
            # value.  With the full 10-block schedule consecutive dependent
            # tasks are >= 10 apart and one-task-per-slot is safe; with
            # wavefront skips the sparse head/tail steps bring them as close
            # as 1-2 apart, so stall slots are inserted here.
            start = []
            writer_end = {}
            cur = -1
            for i, (step, n) in enumerate(tasks):
                lo = cur + 1
                for src in ((n, n - 1) if n > 0 else (n,)):
                    if src in writer_end:
                        lo = max(lo, writer_end[src] + 1)
                start.append(lo)
                cur = lo
                writer_end[n] = lo + NSTAGE - 1
            total_slots = start[-1] + NSTAGE if T else 0

            states = {}
            nxt = 0
            live = []
            for tau in range(total_slots):
                while nxt < T and start[nxt] == tau:
                    states[nxt] = {"task": tasks[nxt], "ps": psr[nxt % 4]}
                    live.append(nxt)
                    nxt += 1
                for i in list(live):
                    j = tau - start[i]
                    if 0 <= j <= 4:
                        emit_phase1(states[i], j)
                for i in live:
                    j = tau - start[i]
                    if 0 <= j < 4:
                        emit_mms(states[i])
                for i in list(live):
                    j = tau - start[i]
                    if 0 <= j < 4:
                        emit_s_t(states[i])
                    elif j == NSTAGE - 1:
                        del states[i]
                        live.remove(i)

            # ---- head: out^T = head_W @ v[9]^T + head_b  (fp16 matmuls) ----
            # head constants are loaded late so the main-loop weight stream
            # gets the DMA engines first at kernel start
            nc.gpsimd.dma_start(headWT_sb[:], headWT[:])
            nc.gpsimd.dma_start(headB_sb[:], headB[:])
            nc.vector.tensor_copy(headWT2[:], headWT_sb[:])
            nc.vector.tensor_copy(headB2[:], headB_sb[:])
            headWT_sb, headB_sb = headWT2, headB2
            outsb = kpool.tile([128, KO * BL], F32, tag="outsb", bufs=1)
            ph = psr[1][0]
            for m in range(KO):
                mc = slice(m * 128, (m + 1) * 128)
                for k in range(KH):
                    nc.tensor.matmul(
                        ph[:, mc],
                        headWT_sb[:, k * DOUT + m * 128 : k * DOUT + (m + 1) * 128],
                        v[NB - 1][:, k * 128 : (k + 1) * 128],
                        start=(k == 0),
                        stop=(k == KH - 1),
                    )
                nc.scalar.activation(
                    outsb[:, m * BL : (m + 1) * BL], ph[:, mc], Ident,
                    bias=headB_sb[:, m : m + 1], scale=1.0,
                )
            nc.sync.dma_start(outT[:], outsb[:])
    nc.compile()
    return nc


def _tile_k(a):
    """[K, M] -> [128, (K//128)*M] laid out (k_lo, (k_hi, m))."""
    K, M = a.shape
    return np.ascontiguousarray(
        a.reshape(K // 128, 128, M).transpose(1, 0, 2).reshape(128, (K // 128) * M)
    )


def kernel(**inputs) -> np.ndarray:
    x = np.asarray(inputs["x"], np.float32)
    embed_W = np.asarray(inputs["embed_W"], np.float32)
    embed_b = np.asarray(inputs["embed_b"], np.float32)
    block_W = np.asarray(inputs["block_W"], np.float32)
    block_b = np.asarray(inputs["block_b"], np.float32)
    head_W = np.asarray(inputs["head_W"], np.float32)
    head_b = np.asarray(inputs["head_b"], np.float32)
    steps = int(np.asarray(inputs["steps"]))

    zero_bias = bool(np.all(block_b == 0.0))

    embWT = _tile_k(embed_W.T)
    headWT = _tile_k(head_W.T.astype(np.float16))
    Wt = block_W.transpose(0, 2, 1) * np.float32(0.5)  # [NB, K=h_in, M=d_out]
    Wa = Wt.astype(np.float16)
    Wb = (2.0 * Wt - Wa.astype(np.float32)).astype(np.float16)
    Wab = np.stack(
        [
            np.stack([_tile_k(Wa[n]) for n in range(NB)]),
            np.stack([_tile_k(Wb[n]) for n in range(NB)]),
        ]
    )  # [2, NB, 128, 8*1024] f16
    embB = np.ascontiguousarray(embed_b.reshape(KH, 128).T)
    # bF[p, n*H + m*128 + col] = block_b[n, m*128 + p]  (broadcast along batch)
    bF = np.ascontiguousarray(
        np.broadcast_to(
            block_b.reshape(NB, KH, 128, 1).transpose(2, 0, 1, 3), (128, NB, KH, BL)
        ).reshape(128, NB * H)
    ).astype(np.float16)
    headB = np.ascontiguousarray(head_b.reshape(KO, 128).T)

    in_maps = []
    for ci in range(NCORES):
        xTc = _tile_k(np.ascontiguousarray(x[ci * BL : (ci + 1) * BL].T))
        in_maps.append(
            dict(xT=xTc, embWT=embWT, embB=embB, Wab=Wab, bF=bF,
                 headWT=headWT, headB=headB)
        )

    nc = build_nc(steps, zero_bias)
    res = run_bass_kernel_spmd(nc, in_maps, core_ids=list(range(NCORES)))

    out = np.empty((B, DOUT), np.float32)
    for ci in range(NCORES):
        oT = res.results[ci]["outT"]  # [128, (do_hi=4, b=128)] = out^T tiled
        out[ci * BL : (ci + 1) * BL] = (
            oT.reshape(128, KO, BL).transpose(2, 1, 0).reshape(BL, DOUT)
        )
    return out



# revision 21
# speedup vs baseline: 1.8874x; 1.0068x over previous
"""Trainium2 Bass kernel for nn_DeepRecursiveNetwork.

Math (reference): 30 outer steps; each step, per block n (0..9):
    inp  = h[n] + block_in[n]           (block_in = x_emb for n=0 else h[n-1] from prev step)
    inner equilibrium, 5 iters from h'=0:
        h' = 0.5 h' + 0.5 tanh(h' @ W[n].T + b[n] + inp)
    h[n] = 0.5 h[n] + 0.5 h'
Output: h[9] @ head_W.T + head_b.

Device formulation (per core, 8-way data parallel over batch, B_local=128):
  - All recurrent tensors live TRANSPOSED in SBUF as [128, 8*128] tiles laid
    out (d_lo, (d_hi, b)) so matmuls (out = lhsT.T @ rhs, contraction on the
    partition dim) need no transposes anywhere.
  - Inner state substitution u = 2*h' with pre-halved weights Wt = W.T/2:
        u_{k+1} = 0.5*u_k + tanh(Wt.T-matmul(u_k) + c + b[n])
    u_1 = tanh(c + b[n]), outer update v[n] = 0.5 v[n] + 0.25 u_5.
  - Matmuls in fp16; two complementary fp16 roundings of the weights are used
    on alternating outer steps so the correlated rounding bias cancels.
    All elementwise math is fp32 internally; PSUM accumulation is fp32.
  - Wavefront skips: with zero block biases, block n's state is exactly zero
    until step n (zeros propagate: tanh(0)=0), and block n's updates after
    step steps-NB+n cannot reach the head output (shortest path to block 9
    takes 9-n steps).  Both skips are exact; 300 -> 210 block-step tasks.
  - Elementwise work uses wide instructions (two [128,512] halves or full
    [128,1024]): per inner round one DVE add (psum+cb), one ACT tanh, one
    DVE axpy (4x-mode ts_mul + 2x-mode f16 tensor_tensor); the cb add runs
    on the otherwise idle gpsimd engine.  State v and activations are fp16;
    pre-activations stay fp32.
  - Tasks (step, block) run as a 5-stage skewed software pipeline
    [setup+round1, round2, round3, round4, v-update] with one new task per
    slot, so the PE sees four consecutive 64-MM batches from four different
    tasks and never drains (any PE idle would also drop the cost-model
    p-state from 2.4 to 1.2 GHz for 3us).  Each round's axpy is emitted one
    slot after its tanh so engine queues never head-of-line block.
  - Per-task start slots respect cross-task RAW order: a task reading v[n]
    is emitted only after the previous writer's final stage, which inserts
    stall slots at the sparse wavefront head/tail (without this the skewed
    emission reads stale state - program order defines the dataflow).
  - PSUM = eight 1-bank [128,512] tiles = four (A,B) region pairs rotating
    across in-flight tasks.
  - Weights (20 MB fp16 per rounding set) stream from HBM per (block, step),
    one [128, 8192] DMA each, quadruple buffered.  Head constants load last.
"""

import numpy as np

import concourse.bacc as bacc
import concourse.mybir as mybir
from concourse.bass_utils import run_bass_kernel_spmd
from concourse.tile import TileContext

F32 = mybir.dt.float32
F16 = mybir.dt.float16

B, DIN, H, DOUT, NB = 1024, 512, 1024, 512, 10
NCORES = 8
BL = B // NCORES  # 128 batch per core
KH = H // 128     # 8 k/m tiles over H
KD = DIN // 128   # 4 k tiles over DIN
KO = DOUT // 128  # 4 m tiles over DOUT
INNER = 5
Tanh = mybir.ActivationFunctionType.Tanh
Ident = mybir.ActivationFunctionType.Identity
MULT = mybir.AluOpType.mult
ADD = mybir.AluOpType.add


def active_blocks(step: int, steps: int, skip_fwd: bool):
    """Blocks whose update at `step` is needed (descending order)."""
    ns = []
    for n in range(NB - 1, -1, -1):
        if skip_fwd and n > step:
            continue  # state still exactly zero
        if step > steps - NB + n:
            continue  # cannot influence block NB-1 by the last step
        ns.append(n)
    return ns


def build_nc(steps: int, zero_bias: bool = True, debug: bool = False, skips: bool = True):
    nc = bacc.Bacc(None, target_bir_lowering=False)
    xT = nc.dram_tensor("xT", [128, KD * BL], F32, kind="ExternalInput")
    embWT = nc.dram_tensor("embWT", [128, KD * H], F32, kind="ExternalInput")
    embB = nc.dram_tensor("embB", [128, KH], F32, kind="ExternalInput")
    Wab = nc.dram_tensor("Wab", [2, NB, 128, KH * H], F16, kind="ExternalInput")
    bF = nc.dram_tensor("bF", [128, NB * H], F16, kind="ExternalInput")
    headWT = nc.dram_tensor("headWT", [128, KH * DOUT], F16, kind="ExternalInput")
    headB = nc.dram_tensor("headB", [128, KO], F32, kind="ExternalInput")
    outT = nc.dram_tensor("outT", [128, KO * BL], F32, kind="ExternalOutput")
    dbg = None
    if debug:
        dbg = nc.dram_tensor("dbg", [128, (NB + 1) * H], F32, kind="ExternalOutput")

    with TileContext(nc) as tc:
        with (
            tc.tile_pool(name="const", bufs=1) as cpool,
            tc.tile_pool(name="state", bufs=1) as spool,
            tc.tile_pool(name="wts", bufs=3) as wpool,
            tc.tile_pool(name="work", bufs=2) as kpool,
            tc.tile_pool(name="psum", bufs=1, space="PSUM") as ppool,
        ):
            # ---- persistent state init first: Pool memsets run during the
            # constant DMAs instead of serializing on the DVE behind them ----
            v = [spool.tile([128, H], F16, tag=f"v{n}", bufs=1, name=f"v{n}")
                 for n in range(NB)]
            xT_sb = cpool.tile([128, KD * BL], F32, tag="xt", bufs=1)
            embWT_sb = cpool.tile([128, KD * H], F32, tag="embwt", bufs=1)
            embB_sb = cpool.tile([128, KH], F32, tag="embb", bufs=1)
            headWT_sb = cpool.tile([128, KH * DOUT], F16, tag="hwt", bufs=1)
            headB_sb = cpool.tile([128, KO], F32, tag="hb", bufs=1)
            # embed constants on the SP (sync) queue: HWDGE enqueue is fast,
            # and program order puts them ahead of the weight stream in the
            # shared DMA FIFO (gpsimd SWDGE enqueue costs ~1us each and loses
            # the race against the weight DMAs)
            nc.sync.dma_start(xT_sb[:], xT[:])
            nc.sync.dma_start(embWT_sb[:], embWT[:])
            nc.sync.dma_start(embB_sb[:], embB[:])
            bF_sb = None
            if not zero_bias:
                bF_sb = cpool.tile([128, NB * H], F16, tag="bf", bufs=1)
                nc.gpsimd.dma_start(bF_sb[:], bF[:])
            for n in range(NB):
                nc.gpsimd.memset(v[n][:], 0.0)
            # No DVE staging: xT/embWT/embB arrive on the same (gpsimd) DMA
            # queue, so the self-loading fp32 embed matmuls' single sync-wait
            # slot covers all three with one semaphore tick.

            # ---- persistent state (transposed layout, fp16) ----
            xemb = spool.tile([128, H], F16, tag="xemb", bufs=1)

            # eight 1-bank PSUM halves -> four (A, B) region pairs
            psh = [ppool.tile([128, 512], F32, tag=f"ps{i}", bufs=1, name=f"ps{i}")
                   for i in range(8)]
            psr = [(psh[2 * i], psh[2 * i + 1]) for i in range(4)]

            # ---- embed: xemb = (x @ embed_W.T + embed_b)^T  (fp32 matmuls) ----
            for m in range(KH):
                pe = psr[0][0] if m < 4 else psr[0][1]
                mc = slice((m % 4) * 128, (m % 4 + 1) * 128)
                for k in range(KD):
                    nc.tensor.matmul(
                        pe[:, mc],
                        embWT_sb[:, k * H + m * 128 : k * H + (m + 1) * 128],
                        xT_sb[:, k * BL : (k + 1) * BL],
                        start=(k == 0),
                        stop=(k == KD - 1),
                    )
                nc.scalar.activation(
                    xemb[:, m * 128 : (m + 1) * 128], pe[:, mc], Ident,
                    bias=embB_sb[:, m : m + 1], scale=1.0,
                )

            if debug:
                dx = kpool.tile([128, H], F32, tag="dx", bufs=1)
                nc.vector.tensor_copy(dx[:], xemb[:])
                nc.sync.dma_start(dbg[:, NB * H : (NB + 1) * H], dx[:])

            # ---- main recurrence (skewed software pipeline) ----
            tasks = []
            for step in range(steps):
                for n in (active_blocks(step, steps, zero_bias) if skips else range(NB - 1, -1, -1)):
                    tasks.append((step, n))

            def emit_mms(st):
                """64-MM batch for the current round (moving operand st["u"])."""
                (psA, psB), wt, u = st["ps"], st["wt"], st["u"]
                for m in range(KH):
                    ph = psA if m < 4 else psB
                    mc = slice((m % 4) * 128, (m % 4 + 1) * 128)
                    for kk in range(KH):
                        nc.tensor.matmul(
                            ph[:, mc],
                            wt[:, kk * H + m * 128 : kk * H + (m + 1) * 128],
                            u[:, kk * 128 : (kk + 1) * 128],
                            start=(kk == 0),
                            stop=(kk == KH - 1),
                        )

            def emit_s_t(st):
                """s = ps + cb; t = tanh(s), split into the two PSUM-bank
                halves so the chain for the first half starts as soon as the
                m=0..3 matmul groups finish (~1.7us before the full batch)."""
                ts_ = []
                for h_ in range(2):
                    hc = slice(h_ * 512, (h_ + 1) * 512)
                    s = kpool.tile([128, 512], F32, tag=f"s{h_}", bufs=(3 if not zero_bias else 4), name="s")
                    nc.vector.tensor_add(s[:], st["ps"][h_][:], st["cb"][:, hc])
                    t = kpool.tile([128, 512], F16, tag=f"t{h_}", bufs=(4 if not zero_bias else 6), name="t")
                    nc.scalar.activation(t[:], s[:], Tanh, bias=0.0, scale=1.0)
                    ts_.append(t)
                st["t"] = ts_

            def emit_axpy(st):
                """u <- 0.5 u + t (previous round's t, already computed)."""
                u, (tA, tB) = st["u"], st["t"]
                # ts_mul runs in the DVE 4x mode, tt-add in 2x: 921ns total
                # vs 1127ns for the fused scalar_tensor_tensor.
                uh = kpool.tile([128, H], F16, tag="uh", bufs=3, name="uh")
                nc.vector.tensor_scalar_mul(uh[:], u[:], 0.5)
                un = kpool.tile([128, H], F16, tag="u", bufs=(6 if not zero_bias else 7), name="un")
                nc.vector.tensor_add(un[:, 0:512], uh[:, 0:512], tA[:])
                nc.vector.tensor_add(un[:, 512:1024], uh[:, 512:1024], tB[:])
                st["u"] = un

            def emit_phase1(st, j):
                """Early work: weight DMA, cb, u1 (j=0); axpy (j>=1); v-update
                (j=4).  All inputs were produced in earlier slots, so these
                never stall the engine queues."""
                step, n = st["task"]
                if j == 0:
                    par = step % 2
                    wt = wpool.tile([128, KH * H], F16, tag="w", bufs=(3 if (debug or not zero_bias) else 4), name="w")
                    nc.sync.dma_start(wt[:], Wab[par, n, :, :])
                    st["wt"] = wt
                    binT = xemb if n == 0 else v[n - 1]
                    cb = kpool.tile([128, H], F16, tag="cb", bufs=(4 if not zero_bias else 5), name="cb")
                    # cb on the (otherwise idle) gpsimd engine
                    nc.gpsimd.tensor_tensor(cb[:], v[n][:], binT[:], ADD)
                    if not zero_bias:
                        cb2 = kpool.tile([128, H], F16, tag="cb2", bufs=3, name="cb2")
                        nc.gpsimd.tensor_tensor(
                            cb2[:], cb[:], bF_sb[:, n * H : (n + 1) * H], ADD
                        )
                        cb = cb2
                    st["cb"] = cb
                    # u1 = tanh(cb)   (inner iter 0; state is zero)
                    u = kpool.tile([128, H], F16, tag="u", bufs=(6 if not zero_bias else 7), name="u")
                    nc.scalar.activation(u[:], cb[:], Tanh, bias=0.0, scale=1.0)
                    st["u"] = u
                else:
                    emit_axpy(st)
                    if j == 4:
                        # v[n] = 0.5 v[n] + 0.25 u5
                        vh = kpool.tile([128, H], F16, tag="vh", bufs=2, name="vh")
                        nc.vector.tensor_scalar_mul(vh[:], v[n][:], 0.5)
                        uq = kpool.tile([128, H], F16, tag="uq", bufs=2, name="uq")
                        nc.vector.tensor_scalar_mul(uq[:], st["u"][:], 0.25)
                        nc.vector.tensor_add(v[n][:], vh[:], uq[:])

            NSTAGE = 5
            T = len(tasks)
            # Per-task start slots.  A task READS v[n] and v[n-1] at its
            # first stage (S0) and WRITES v[n] at its last (S4).  Emission
            # (= program) order must put every read of v[k] after the
            # previous writer's S4, otherwise the reader picks up the stale
            # value.  With the full 10-block schedule consecutive dependent
            # tasks are >= 10 apart and one-task-per-slot is safe; with
            # wavefront skips the sparse head/tail steps bring them as close
            # as 1-2 apart, so stall slots are inserted here.
            start = []
            writer_end = {}
            cur = -1
            for i, (step, n) in enumerate(tasks):
                lo = cur + 1
                for src in ((n, n - 1) if n > 0 else (n,)):
                    if src in writer_end:
                        lo = max(lo, writer_end[src] + 1)
                start.append(lo)
                cur = lo
                writer_end[n] = lo + NSTAGE - 1
            total_slots = start[-1] + NSTAGE if T else 0

            states = {}
            nxt = 0
            live = []
            for tau in range(total_slots):
                while nxt < T and start[nxt] == tau:
                    states[nxt] = {"task": tasks[nxt], "ps": psr[nxt % 4]}
                    live.append(nxt)
                    nxt += 1
                for i in list(live):
                    j = tau - start[i]
                    if 0 <= j <= 4:
                        emit_phase1(states[i], j)
                for i in live:
                    j = tau - start[i]
                    if 0 <= j < 4:
                        emit_mms(states[i])
                for i in list(live):
                    j = tau - start[i]
                    if 0 <= j < 4:
                        emit_s_t(states[i])
                    elif j == NSTAGE - 1:
                        del states[i]
                        live.remove(i)

            # ---- head: out^T = head_W @ v[9]^T + head_b  (fp16 matmuls) ----
            # head constants are loaded late so the main-loop weight stream
            # gets the DMA engines first at kernel start
            nc.gpsimd.dma_start(headWT_sb[:], headWT[:])
            nc.gpsimd.dma_start(headB_sb[:], headB[:])
            outsb = kpool.tile([128, KO * BL], F32, tag="outsb", bufs=1)
            ph = psr[1][0]
            for m in range(KO):
                mc = slice(m * 128, (m + 1) * 128)
                for k in range(KH):
                    nc.tensor.matmul(
                        ph[:, mc],
                        headWT_sb[:, k * DOUT + m * 128 : k * DOUT + (m + 1) * 128],
                        v[NB - 1][:, k * 128 : (k + 1) * 128],
                        start=(k == 0),
                        stop=(k == KH - 1),
                    )
                nc.scalar.activation(
                    outsb[:, m * BL : (m + 1) * BL], ph[:, mc], Ident,
                    bias=headB_sb[:, m : m + 1], scale=1.0,
                )
            nc.sync.dma_start(outT[:], outsb[:])
    nc.compile()
    return nc


def _tile_k(a):
    """[K, M] -> [128, (K//128)*M] laid out (k_lo, (k_hi, m))."""
    K, M = a.shape
    return np.ascontiguousarray(
        a.reshape(K // 128, 128, M).transpose(1, 0, 2).reshape(128, (K // 128) * M)
    )


def kernel(**inputs) -> np.ndarray:
    x = np.asarray(inputs["x"], np.float32)
    embed_W = np.asarray(inputs["embed_W"], np.float32)
    embed_b = np.asarray(inputs["embed_b"], np.float32)
    block_W = np.asarray(inputs["block_W"], np.float32)
    block_b = np.asarray(inputs["block_b"], np.float32)
    head_W = np.asarray(inputs["head_W"], np.float32)
    head_b = np.asarray(inputs["head_b"], np.float32)
    steps = int(np.asarray(inputs["steps"]))

    zero_bias = bool(np.all(block_b == 0.0))

    embWT = _tile_k(embed_W.T)
    headWT = _tile_k(head_W.T.astype(np.float16))
    Wt = block_W.transpose(0, 2, 1) * np.float32(0.5)  # [NB, K=h_in, M=d_out]
    Wa = Wt.astype(np.float16)
    Wb = (2.0 * Wt - Wa.astype(np.float32)).astype(np.float16)
    Wab = np.stack(
        [
            np.stack([_tile_k(Wa[n]) for n in range(NB)]),
            np.stack([_tile_k(Wb[n]) for n in range(NB)]),
        ]
    )  # [2, NB, 128, 8*1024] f16
    embB = np.ascontiguousarray(embed_b.reshape(KH, 128).T)
    # bF[p, n*H + m*128 + col] = block_b[n, m*128 + p]  (broadcast along batch)
    bF = np.ascontiguousarray(
        np.broadcast_to(
            block_b.reshape(NB, KH, 128, 1).transpose(2, 0, 1, 3), (128, NB, KH, BL)
        ).reshape(128, NB * H)
    ).astype(np.float16)
    headB = np.ascontiguousarray(head_b.reshape(KO, 128).T)

    in_maps = []
    for ci in range(NCORES):
        xTc = _tile_k(np.ascontiguousarray(x[ci * BL : (ci + 1) * BL].T))
        in_maps.append(
            dict(xT=xTc, embWT=embWT, embB=embB, Wab=Wab, bF=bF,
                 headWT=headWT, headB=headB)
        )

    nc = build_nc(steps, zero_bias)
    res = run_bass_kernel_spmd(nc, in_maps, core_ids=list(range(NCORES)))

    out = np.empty((B, DOUT), np.float32)
    for ci in range(NCORES):
        oT = res.results[ci]["outT"]  # [128, (do_hi=4, b=128)] = out^T tiled
        out[ci * BL : (ci + 1) * BL] = (
            oT.reshape(128, KO, BL).transpose(2, 1, 0).reshape(BL, DOUT)
        )
    return out



# revision 22
# speedup vs baseline: 1.8960x; 1.0045x over previous
"""Trainium2 Bass kernel for nn_DeepRecursiveNetwork.

Math (reference): 30 outer steps; each step, per block n (0..9):
    inp  = h[n] + block_in[n]           (block_in = x_emb for n=0 else h[n-1] from prev step)
    inner equilibrium, 5 iters from h'=0:
        h' = 0.5 h' + 0.5 tanh(h' @ W[n].T + b[n] + inp)
    h[n] = 0.5 h[n] + 0.5 h'
Output: h[9] @ head_W.T + head_b.

Device formulation (per core, 8-way data parallel over batch, B_local=128):
  - All recurrent tensors live TRANSPOSED in SBUF as [128, 8*128] tiles laid
    out (d_lo, (d_hi, b)) so matmuls (out = lhsT.T @ rhs, contraction on the
    partition dim) need no transposes anywhere.
  - Inner state substitution u = 2*h' with pre-halved weights Wt = W.T/2:
        u_{k+1} = 0.5*u_k + tanh(Wt.T-matmul(u_k) + c + b[n])
    u_1 = tanh(c + b[n]), outer update v[n] = 0.5 v[n] + 0.25 u_5.
  - Matmuls in fp16; two complementary fp16 roundings of the weights are used
    on alternating outer steps so the correlated rounding bias cancels.
    All elementwise math is fp32 internally; PSUM accumulation is fp32.
  - Wavefront skips: with zero block biases, block n's state is exactly zero
    until step n (zeros propagate: tanh(0)=0), and block n's updates after
    step steps-NB+n cannot reach the head output (shortest path to block 9
    takes 9-n steps).  Both skips are exact; 300 -> 210 block-step tasks.
  - Elementwise work uses wide instructions (two [128,512] halves or full
    [128,1024]): per inner round one DVE add (psum+cb), one ACT tanh, one
    DVE axpy (4x-mode ts_mul + 2x-mode f16 tensor_tensor); the cb add runs
    on the otherwise idle gpsimd engine.  State v and activations are fp16;
    pre-activations stay fp32.
  - Tasks (step, block) run as a 5-stage skewed software pipeline
    [setup+round1, round2, round3, round4, v-update] with one new task per
    slot, so the PE sees four consecutive 64-MM batches from four different
    tasks and never drains (any PE idle would also drop the cost-model
    p-state from 2.4 to 1.2 GHz for 3us).  Each round's axpy is emitted one
    slot after its tanh so engine queues never head-of-line block.
  - Per-task start slots respect cross-task RAW order: a task reading v[n]
    is emitted only after the previous writer's final stage, which inserts
    stall slots at the sparse wavefront head/tail (without this the skewed
    emission reads stale state - program order defines the dataflow).
  - PSUM = eight 1-bank [128,512] tiles = four (A,B) region pairs rotating
    across in-flight tasks.
  - Weights (20 MB fp16 per rounding set) stream from HBM per (block, step),
    one [128, 8192] DMA each, quadruple buffered.  Head constants load last.
"""

import numpy as np

import concourse.bacc as bacc
import concourse.mybir as mybir
from concourse.bass_utils import run_bass_kernel_spmd
from concourse.tile import TileContext

F32 = mybir.dt.float32
F16 = mybir.dt.float16

B, DIN, H, DOUT, NB = 1024, 512, 1024, 512, 10
NCORES = 8
BL = B // NCORES  # 128 batch per core
KH = H // 128     # 8 k/m tiles over H
KD = DIN // 128   # 4 k tiles over DIN
KO = DOUT // 128  # 4 m tiles over DOUT
INNER = 5
Tanh = mybir.ActivationFunctionType.Tanh
Ident = mybir.ActivationFunctionType.Identity
MULT = mybir.AluOpType.mult
ADD = mybir.AluOpType.add


def active_blocks(step: int, steps: int, skip_fwd: bool):
    """Blocks whose update at `step` is needed (descending order)."""
    ns = []
    for n in range(NB - 1, -1, -1):
        if skip_fwd and n > step:
            continue  # state still exactly zero
        if step > steps - NB + n:
            continue  # cannot influence block NB-1 by the last step
        ns.append(n)
    return ns


def build_nc(steps: int, zero_bias: bool = True, debug: bool = False, skips: bool = True):
    nc = bacc.Bacc(None, target_bir_lowering=False)
    xT = nc.dram_tensor("xT", [128, KD * BL], F32, kind="ExternalInput")
    embWT = nc.dram_tensor("embWT", [128, KD * H], F32, kind="ExternalInput")
    embB = nc.dram_tensor("embB", [128, KH], F32, kind="ExternalInput")
    Wab = nc.dram_tensor("Wab", [2, NB, 128, KH * H], F16, kind="ExternalInput")
    bF = nc.dram_tensor("bF", [128, NB * H], F16, kind="ExternalInput")
    headWT = nc.dram_tensor("headWT", [128, KH * DOUT], F16, kind="ExternalInput")
    headB = nc.dram_tensor("headB", [128, KO], F32, kind="ExternalInput")
    outT = nc.dram_tensor("outT", [128, KO * BL], F32, kind="ExternalOutput")
    dbg = None
    if debug:
        dbg = nc.dram_tensor("dbg", [128, (NB + 1) * H], F32, kind="ExternalOutput")

    with TileContext(nc) as tc:
        with (
            tc.tile_pool(name="const", bufs=1) as cpool,
            tc.tile_pool(name="state", bufs=1) as spool,
            tc.tile_pool(name="wts", bufs=3) as wpool,
            tc.tile_pool(name="work", bufs=2) as kpool,
            tc.tile_pool(name="psum", bufs=1, space="PSUM") as ppool,
        ):
            # ---- persistent state init first: Pool memsets run during the
            # constant DMAs instead of serializing on the DVE behind them ----
            v = [spool.tile([128, H], F16, tag=f"v{n}", bufs=1, name=f"v{n}")
                 for n in range(NB)]
            xT_sb = cpool.tile([128, KD * BL], F32, tag="xt", bufs=1)
            embWT_sb = cpool.tile([128, KD * H], F32, tag="embwt", bufs=1)
            embB_sb = cpool.tile([128, KH], F32, tag="embb", bufs=1)
            headWT_sb = cpool.tile([128, KH * DOUT], F16, tag="hwt", bufs=1)
            headB_sb = cpool.tile([128, KO], F32, tag="hb", bufs=1)
            # embed constants on the SP (sync) queue: HWDGE enqueue is fast,
            # and program order puts them ahead of the weight stream in the
            # shared DMA FIFO (gpsimd SWDGE enqueue costs ~1us each and loses
            # the race against the weight DMAs)
            nc.sync.dma_start(xT_sb[:], xT[:])
            nc.sync.dma_start(embWT_sb[:], embWT[:])
            nc.sync.dma_start(embB_sb[:], embB[:])
            bF_sb = None
            if not zero_bias:
                bF_sb = cpool.tile([128, NB * H], F16, tag="bf", bufs=1)
                nc.gpsimd.dma_start(bF_sb[:], bF[:])
            for n in range(NB):
                nc.gpsimd.memset(v[n][:], 0.0)
            # No DVE staging: xT/embWT/embB arrive on the same (gpsimd) DMA
            # queue, so the self-loading fp32 embed matmuls' single sync-wait
            # slot covers all three with one semaphore tick.

            # ---- persistent state (transposed layout, fp16) ----
            xemb = spool.tile([128, H], F16, tag="xemb", bufs=1)

            # eight 1-bank PSUM halves -> four (A, B) region pairs
            psh = [ppool.tile([128, 512], F32, tag=f"ps{i}", bufs=1, name=f"ps{i}")
                   for i in range(8)]
            psr = [(psh[2 * i], psh[2 * i + 1]) for i in range(4)]

            # ---- embed: xemb = (x @ embed_W.T + embed_b)^T  (fp32 matmuls) ----
            for m in range(KH):
                pe = psr[0][0] if m < 4 else psr[0][1]
                mc = slice((m % 4) * 128, (m % 4 + 1) * 128)
                for k in range(KD):
                    nc.tensor.matmul(
                        pe[:, mc],
                        embWT_sb[:, k * H + m * 128 : k * H + (m + 1) * 128],
                        xT_sb[:, k * BL : (k + 1) * BL],
                        start=(k == 0),
                        stop=(k == KD - 1),
                    )
                nc.scalar.activation(
                    xemb[:, m * 128 : (m + 1) * 128], pe[:, mc], Ident,
                    bias=embB_sb[:, m : m + 1], scale=1.0,
                )

            if debug:
                dx = kpool.tile([128, H], F32, tag="dx", bufs=1)
                nc.vector.tensor_copy(dx[:], xemb[:])
                nc.sync.dma_start(dbg[:, NB * H : (NB + 1) * H], dx[:])

            # ---- main recurrence (skewed software pipeline) ----
            tasks = []
            for step in range(steps):
                for n in (active_blocks(step, steps, zero_bias) if skips else range(NB - 1, -1, -1)):
                    tasks.append((step, n))

            def emit_mms(st):
                """64-MM batch for the current round (moving operand st["u"])."""
                (psA, psB), wt, u = st["ps"], st["wt"], st["u"]
                for m in range(KH):
                    ph = psA if m < 4 else psB
                    mc = slice((m % 4) * 128, (m % 4 + 1) * 128)
                    for kk in range(KH):
                        nc.tensor.matmul(
                            ph[:, mc],
                            wt[:, kk * H + m * 128 : kk * H + (m + 1) * 128],
                            u[:, kk * 128 : (kk + 1) * 128],
                            start=(kk == 0),
                            stop=(kk == KH - 1),
                        )

            def emit_s_t(st):
                """s = ps + cb; t = tanh(s), split into the two PSUM-bank
                halves so the chain for the first half starts as soon as the
                m=0..3 matmul groups finish (~1.7us before the full batch)."""
                ts_ = []
                for h_ in range(2):
                    hc = slice(h_ * 512, (h_ + 1) * 512)
                    s = kpool.tile([128, 512], F32, tag=f"s{h_}", bufs=(3 if not zero_bias else 4), name="s")
                    nc.vector.tensor_add(s[:], st["ps"][h_][:], st["cb"][:, hc])
                    t = kpool.tile([128, 512], F16, tag=f"t{h_}", bufs=(4 if not zero_bias else 6), name="t")
                    nc.scalar.activation(t[:], s[:], Tanh, bias=0.0, scale=1.0)
                    ts_.append(t)
                st["t"] = ts_

            def emit_axpy(st):
                """u <- 0.5 u + t (previous round's t, already computed)."""
                u, (tA, tB) = st["u"], st["t"]
                # ts_mul runs in the DVE 4x mode, tt-add in 2x: 921ns total
                # vs 1127ns for the fused scalar_tensor_tensor.
                uh = kpool.tile([128, H], F16, tag="uh", bufs=3, name="uh")
                nc.vector.tensor_scalar_mul(uh[:], u[:], 0.5)
                un = kpool.tile([128, H], F16, tag="u", bufs=(6 if not zero_bias else 7), name="un")
                nc.vector.tensor_add(un[:, 0:512], uh[:, 0:512], tA[:])
                nc.vector.tensor_add(un[:, 512:1024], uh[:, 512:1024], tB[:])
                st["u"] = un

            def emit_phase1(st, j):
                """Early work: weight DMA, cb, u1 (j=0); axpy (j>=1); v-update
                (j=4).  All inputs were produced in earlier slots, so these
                never stall the engine queues."""
                step, n = st["task"]
                if j == 0:
                    par = step % 2
                    wt = wpool.tile([128, KH * H], F16, tag="w", bufs=(3 if (debug or not zero_bias) else 4), name="w")
                    nc.sync.dma_start(wt[:], Wab[par, n, :, :])
                    st["wt"] = wt
                    binT = xemb if n == 0 else v[n - 1]
                    cb = kpool.tile([128, H], F16, tag="cb", bufs=(4 if not zero_bias else 5), name="cb")
                    # cb on the (otherwise idle) gpsimd engine
                    nc.gpsimd.tensor_tensor(cb[:], v[n][:], binT[:], ADD)
                    if not zero_bias:
                        cb2 = kpool.tile([128, H], F16, tag="cb2", bufs=3, name="cb2")
                        nc.gpsimd.tensor_tensor(
                            cb2[:], cb[:], bF_sb[:, n * H : (n + 1) * H], ADD
                        )
                        cb = cb2
                    st["cb"] = cb
                    # u1 = tanh(cb)   (inner iter 0; state is zero)
                    u = kpool.tile([128, H], F16, tag="u", bufs=(6 if not zero_bias else 7), name="u")
                    nc.scalar.activation(u[:], cb[:], Tanh, bias=0.0, scale=1.0)
                    st["u"] = u
                else:
                    emit_axpy(st)
                    if j == 4:
                        # v[n] = 0.5 v[n] + 0.25 u5
                        vh = kpool.tile([128, H], F16, tag="vh", bufs=2, name="vh")
                        nc.vector.tensor_scalar_mul(vh[:], v[n][:], 0.5)
                        uq = kpool.tile([128, H], F16, tag="uq", bufs=2, name="uq")
                        nc.vector.tensor_scalar_mul(uq[:], st["u"][:], 0.25)
                        nc.vector.tensor_add(v[n][:], vh[:], uq[:])

            NSTAGE = 5
            T = len(tasks)
            # Per-task start slots.  A task READS v[n] and v[n-1] at its
            # first stage (S0) and WRITES v[n] at its last (S4).  Emission
            # (= program) order must put every read of v[k] after the
            # previous writer's S4, otherwise the reader picks up the stale
            # value.  With the full 10-block schedule consecutive dependent
            # tasks are >= 10 apart and one-task-per-slot is safe; with
            # wavefront skips the sparse head/tail steps bring them as close
            # as 1-2 apart, so stall slots are inserted here.
            start = []
            writer_end = {}
            cur = -1
            for i, (step, n) in enumerate(tasks):
                lo = cur + 1
                for src in ((n, n - 1) if n > 0 else (n,)):
                    if src in writer_end:
                        # same-slot is safe: phase-1 emission is oldest-task
                        # first, so the writer's S4 v-update precedes this
                        # task's S0 read in program order within one slot
                        lo = max(lo, writer_end[src])
                start.append(lo)
                cur = lo
                writer_end[n] = lo + NSTAGE - 1
            total_slots = start[-1] + NSTAGE if T else 0

            states = {}
            nxt = 0
            live = []
            for tau in range(total_slots):
                while nxt < T and start[nxt] == tau:
                    states[nxt] = {"task": tasks[nxt], "ps": psr[nxt % 4]}
                    live.append(nxt)
                    nxt += 1
                for i in list(live):
                    j = tau - start[i]
                    if 0 <= j <= 4:
                        emit_phase1(states[i], j)
                for i in live:
                    j = tau - start[i]
                    if 0 <= j < 4:
                        emit_mms(states[i])
                for i in list(live):
                    j = tau - start[i]
                    if 0 <= j < 4:
                        emit_s_t(states[i])
                    elif j == NSTAGE - 1:
                        del states[i]
                        live.remove(i)

            # ---- head: out^T = head_W @ v[9]^T + head_b  (fp16 matmuls) ----
            # head constants are loaded late so the main-loop weight stream
            # gets the DMA engines first at kernel start
            nc.gpsimd.dma_start(headWT_sb[:], headWT[:])
            nc.gpsimd.dma_start(headB_sb[:], headB[:])
            outsb = kpool.tile([128, KO * BL], F32, tag="outsb", bufs=1)
            ph = psr[1][0]
            for m in range(KO):
                mc = slice(m * 128, (m + 1) * 128)
                for k in range(KH):
                    nc.tensor.matmul(
                        ph[:, mc],
                        headWT_sb[:, k * DOUT + m * 128 : k * DOUT + (m + 1) * 128],
                        v[NB - 1][:, k * 128 : (k + 1) * 128],
                        start=(k == 0),
                        stop=(k == KH - 1),
                    )
                nc.scalar.activation(
                    outsb[:, m * BL : (m + 1) * BL], ph[:, mc], Ident,
                    bias=headB_sb[:, m : m + 1], scale=1.0,
                )
            nc.sync.dma_start(outT[:], outsb[:])
    nc.compile()
    return nc


def _tile_k(a):
    """[K, M] -> [128, (K//128)*M] laid out (k_lo, (k_hi, m))."""
    K, M = a.shape
    return np.ascontiguousarray(
        a.reshape(K // 128, 128, M).transpose(1, 0, 2).reshape(128, (K // 128) * M)
    )


def kernel(**inputs) -> np.ndarray:
    x = np.asarray(inputs["x"], np.float32)
    embed_W = np.asarray(inputs["embed_W"], np.float32)
    embed_b = np.asarray(inputs["embed_b"], np.float32)
    block_W = np.asarray(inputs["block_W"], np.float32)
    block_b = np.asarray(inputs["block_b"], np.float32)
    head_W = np.asarray(inputs["head_W"], np.float32)
    head_b = np.asarray(inputs["head_b"], np.float32)
    steps = int(np.asarray(inputs["steps"]))

    zero_bias = bool(np.all(block_b == 0.0))

    embWT = _tile_k(embed_W.T)
    headWT = _tile_k(head_W.T.astype(np.float16))
    Wt = block_W.transpose(0, 2, 1) * np.float32(0.5)  # [NB, K=h_in, M=d_out]
    Wa = Wt.astype(np.float16)
    Wb = (2.0 * Wt - Wa.astype(np.float32)).astype(np.float16)
    Wab = np.stack(
        [
            np.stack([_tile_k(Wa[n]) for n in range(NB)]),
            np.stack([_tile_k(Wb[n]) for n in range(NB)]),
        ]
    )  # [2, NB, 128, 8*1024] f16
    embB = np.ascontiguousarray(embed_b.reshape(KH, 128).T)
    # bF[p, n*H + m*128 + col] = block_b[n, m*128 + p]  (broadcast along batch)
    bF = np.ascontiguousarray(
        np.broadcast_to(
            block_b.reshape(NB, KH, 128, 1).transpose(2, 0, 1, 3), (128, NB, KH, BL)
        ).reshape(128, NB * H)
    ).astype(np.float16)
    headB = np.ascontiguousarray(head_b.reshape(KO, 128).T)

    in_maps = []
    for ci in range(NCORES):
        xTc = _tile_k(np.ascontiguousarray(x[ci * BL : (ci + 1) * BL].T))
        in_maps.append(
            dict(xT=xTc, embWT=embWT, embB=embB, Wab=Wab, bF=bF,
                 headWT=headWT, headB=headB)
        )

    nc = build_nc(steps, zero_bias)
    res = run_bass_kernel_spmd(nc, in_maps, core_ids=list(range(NCORES)))

    out = np.empty((B, DOUT), np.float32)
    for ci in range(NCORES):
        oT = res.results[ci]["outT"]  # [128, (do_hi=4, b=128)] = out^T tiled
        out[ci * BL : (ci + 1) * BL] = (
            oT.reshape(128, KO, BL).transpose(2, 1, 0).reshape(BL, DOUT)
        )
    return out



# revision 23
# speedup vs baseline: 1.8998x; 1.0020x over previous
"""Trainium2 Bass kernel for nn_DeepRecursiveNetwork.

Math (reference): 30 outer steps; each step, per block n (0..9):
    inp  = h[n] + block_in[n]           (block_in = x_emb for n=0 else h[n-1] from prev step)
    inner equilibrium, 5 iters from h'=0:
        h' = 0.5 h' + 0.5 tanh(h' @ W[n].T + b[n] + inp)
    h[n] = 0.5 h[n] + 0.5 h'
Output: h[9] @ head_W.T + head_b.

Device formulation (per core, 8-way data parallel over batch, B_local=128):
  - All recurrent tensors live TRANSPOSED in SBUF as [128, 8*128] tiles laid
    out (d_lo, (d_hi, b)) so matmuls (out = lhsT.T @ rhs, contraction on the
    partition dim) need no transposes anywhere.
  - Inner state substitution u = 2*h' with pre-halved weights Wt = W.T/2:
        u_{k+1} = 0.5*u_k + tanh(Wt.T-matmul(u_k) + c + b[n])
    u_1 = tanh(c + b[n]), outer update v[n] = 0.5 v[n] + 0.25 u_5.
  - Matmuls in fp16; two complementary fp16 roundings of the weights are used
    on alternating outer steps so the correlated rounding bias cancels.
    All elementwise math is fp32 internally; PSUM accumulation is fp32.
  - Wavefront skips: with zero block biases, block n's state is exactly zero
    until step n (zeros propagate: tanh(0)=0), and block n's updates after
    step steps-NB+n cannot reach the head output (shortest path to block 9
    takes 9-n steps).  Both skips are exact; 300 -> 210 block-step tasks.
  - Elementwise work uses wide instructions (two [128,512] halves or full
    [128,1024]): per inner round one DVE add (psum+cb), one ACT tanh, one
    DVE axpy (4x-mode ts_mul + 2x-mode f16 tensor_tensor); the cb add runs
    on the otherwise idle gpsimd engine.  State v and activations are fp16;
    pre-activations stay fp32.
  - Tasks (step, block) run as a 5-stage skewed software pipeline
    [setup+round1, round2, round3, round4, v-update] with one new task per
    slot, so the PE sees four consecutive 64-MM batches from four different
    tasks and never drains (any PE idle would also drop the cost-model
    p-state from 2.4 to 1.2 GHz for 3us).  Each round's axpy is emitted one
    slot after its tanh so engine queues never head-of-line block.
  - Per-task start slots respect cross-task RAW order: a task reading v[n]
    is emitted only after the previous writer's final stage, which inserts
    stall slots at the sparse wavefront head/tail (without this the skewed
    emission reads stale state - program order defines the dataflow).
  - PSUM = eight 1-bank [128,512] tiles = four (A,B) region pairs rotating
    across in-flight tasks.
  - Weights (20 MB fp16 per rounding set) stream from HBM per (block, step),
    one [128, 8192] DMA each, quadruple buffered.  Head constants load last.
"""

import numpy as np

import concourse.bacc as bacc
import concourse.mybir as mybir
from concourse.bass_utils import run_bass_kernel_spmd
from concourse.tile import TileContext

F32 = mybir.dt.float32
F16 = mybir.dt.float16

B, DIN, H, DOUT, NB = 1024, 512, 1024, 512, 10
NCORES = 8
BL = B // NCORES  # 128 batch per core
KH = H // 128     # 8 k/m tiles over H
KD = DIN // 128   # 4 k tiles over DIN
KO = DOUT // 128  # 4 m tiles over DOUT
INNER = 5
Tanh = mybir.ActivationFunctionType.Tanh
Ident = mybir.ActivationFunctionType.Identity
MULT = mybir.AluOpType.mult
ADD = mybir.AluOpType.add


def active_blocks(step: int, steps: int, skip_fwd: bool):
    """Blocks whose update at `step` is needed (descending order)."""
    ns = []
    for n in range(NB - 1, -1, -1):
        if skip_fwd and n > step:
            continue  # state still exactly zero
        if step > steps - NB + n:
            continue  # cannot influence block NB-1 by the last step
        ns.append(n)
    return ns


def build_nc(steps: int, zero_bias: bool = True, debug: bool = False, skips: bool = True):
    nc = bacc.Bacc(None, target_bir_lowering=False)
    xT = nc.dram_tensor("xT", [128, KD * BL], F16, kind="ExternalInput")
    embWT = nc.dram_tensor("embWT", [128, 2 * KD * H], F16, kind="ExternalInput")
    embB = nc.dram_tensor("embB", [128, KH], F32, kind="ExternalInput")
    Wab = nc.dram_tensor("Wab", [2, NB, 128, KH * H], F16, kind="ExternalInput")
    bF = nc.dram_tensor("bF", [128, NB * H], F16, kind="ExternalInput")
    headWT = nc.dram_tensor("headWT", [128, KH * DOUT], F16, kind="ExternalInput")
    headB = nc.dram_tensor("headB", [128, KO], F32, kind="ExternalInput")
    outT = nc.dram_tensor("outT", [128, KO * BL], F32, kind="ExternalOutput")
    dbg = None
    if debug:
        dbg = nc.dram_tensor("dbg", [128, (NB + 1) * H], F32, kind="ExternalOutput")

    with TileContext(nc) as tc:
        with (
            tc.tile_pool(name="const", bufs=1) as cpool,
            tc.tile_pool(name="state", bufs=1) as spool,
            tc.tile_pool(name="wts", bufs=3) as wpool,
            tc.tile_pool(name="work", bufs=2) as kpool,
            tc.tile_pool(name="psum", bufs=1, space="PSUM") as ppool,
        ):
            # ---- persistent state init first: Pool memsets run during the
            # constant DMAs instead of serializing on the DVE behind them ----
            v = [spool.tile([128, H], F16, tag=f"v{n}", bufs=1, name=f"v{n}")
                 for n in range(NB)]
            xT_sb = cpool.tile([128, KD * BL], F16, tag="xt", bufs=1)
            embWT_sb = cpool.tile([128, 2 * KD * H], F16, tag="embwt", bufs=1)
            embB_sb = cpool.tile([128, KH], F32, tag="embb", bufs=1)
            headWT_sb = cpool.tile([128, KH * DOUT], F16, tag="hwt", bufs=1)
            headB_sb = cpool.tile([128, KO], F32, tag="hb", bufs=1)
            # embed constants on the SP (sync) queue: HWDGE enqueue is fast,
            # and program order puts them ahead of the weight stream in the
            # shared DMA FIFO (gpsimd SWDGE enqueue costs ~1us each and loses
            # the race against the weight DMAs)
            nc.sync.dma_start(xT_sb[:], xT[:])
            nc.sync.dma_start(embWT_sb[:], embWT[:])
            nc.sync.dma_start(embB_sb[:], embB[:])
            bF_sb = None
            if not zero_bias:
                bF_sb = cpool.tile([128, NB * H], F16, tag="bf", bufs=1)
                nc.gpsimd.dma_start(bF_sb[:], bF[:])
            for n in range(NB):
                nc.gpsimd.memset(v[n][:], 0.0)
            # No DVE staging: xT/embWT/embB arrive on the same (gpsimd) DMA
            # queue, so the self-loading fp32 embed matmuls' single sync-wait
            # slot covers all three with one semaphore tick.

            # ---- persistent state (transposed layout, fp16) ----
            xemb = spool.tile([128, H], F16, tag="xemb", bufs=1)

            # eight 1-bank PSUM halves -> four (A, B) region pairs
            psh = [ppool.tile([128, 512], F32, tag=f"ps{i}", bufs=1, name=f"ps{i}")
                   for i in range(8)]
            psr = [(psh[2 * i], psh[2 * i + 1]) for i in range(4)]

            # ---- embed: xemb = (x @ embed_W.T + embed_b)^T  (fp32 matmuls) ----
            for m in range(KH):
                pe = psr[0][0] if m < 4 else psr[0][1]
                mc = slice((m % 4) * 128, (m % 4 + 1) * 128)
                for g in range(2 * KD):   # two pre-halved f16 rounding sets
                    q, k = divmod(g, KD)
                    nc.tensor.matmul(
                        pe[:, mc],
                        embWT_sb[:, (q * KD + k) * H + m * 128 : (q * KD + k) * H + (m + 1) * 128],
                        xT_sb[:, k * BL : (k + 1) * BL],
                        start=(g == 0),
                        stop=(g == 2 * KD - 1),
                    )
                nc.scalar.activation(
                    xemb[:, m * 128 : (m + 1) * 128], pe[:, mc], Ident,
                    bias=embB_sb[:, m : m + 1], scale=1.0,
                )

            if debug:
                dx = kpool.tile([128, H], F32, tag="dx", bufs=1)
                nc.vector.tensor_copy(dx[:], xemb[:])
                nc.sync.dma_start(dbg[:, NB * H : (NB + 1) * H], dx[:])

            # ---- main recurrence (skewed software pipeline) ----
            tasks = []
            for step in range(steps):
                for n in (active_blocks(step, steps, zero_bias) if skips else range(NB - 1, -1, -1)):
                    tasks.append((step, n))

            def emit_mms(st):
                """64-MM batch for the current round (moving operand st["u"])."""
                (psA, psB), wt, u = st["ps"], st["wt"], st["u"]
                for m in range(KH):
                    ph = psA if m < 4 else psB
                    mc = slice((m % 4) * 128, (m % 4 + 1) * 128)
                    for kk in range(KH):
                        nc.tensor.matmul(
                            ph[:, mc],
                            wt[:, kk * H + m * 128 : kk * H + (m + 1) * 128],
                            u[:, kk * 128 : (kk + 1) * 128],
                            start=(kk == 0),
                            stop=(kk == KH - 1),
                        )

            def emit_s_t(st):
                """s = ps + cb; t = tanh(s), split into the two PSUM-bank
                halves so the chain for the first half starts as soon as the
                m=0..3 matmul groups finish (~1.7us before the full batch)."""
                ts_ = []
                for h_ in range(2):
                    hc = slice(h_ * 512, (h_ + 1) * 512)
                    s = kpool.tile([128, 512], F32, tag=f"s{h_}", bufs=(3 if not zero_bias else 4), name="s")
                    nc.vector.tensor_add(s[:], st["ps"][h_][:], st["cb"][:, hc])
                    t = kpool.tile([128, 512], F16, tag=f"t{h_}", bufs=(4 if not zero_bias else 6), name="t")
                    nc.scalar.activation(t[:], s[:], Tanh, bias=0.0, scale=1.0)
                    ts_.append(t)
                st["t"] = ts_

            def emit_axpy(st):
                """u <- 0.5 u + t (previous round's t, already computed)."""
                u, (tA, tB) = st["u"], st["t"]
                # ts_mul runs in the DVE 4x mode, tt-add in 2x: 921ns total
                # vs 1127ns for the fused scalar_tensor_tensor.
                uh = kpool.tile([128, H], F16, tag="uh", bufs=3, name="uh")
                nc.vector.tensor_scalar_mul(uh[:], u[:], 0.5)
                un = kpool.tile([128, H], F16, tag="u", bufs=(6 if not zero_bias else 7), name="un")
                nc.vector.tensor_add(un[:, 0:512], uh[:, 0:512], tA[:])
                nc.vector.tensor_add(un[:, 512:1024], uh[:, 512:1024], tB[:])
                st["u"] = un

            def emit_phase1(st, j):
                """Early work: weight DMA, cb, u1 (j=0); axpy (j>=1); v-update
                (j=4).  All inputs were produced in earlier slots, so these
                never stall the engine queues."""
                step, n = st["task"]
                if j == 0:
                    par = step % 2
                    wt = wpool.tile([128, KH * H], F16, tag="w", bufs=(3 if (debug or not zero_bias) else 4), name="w")
                    nc.sync.dma_start(wt[:], Wab[par, n, :, :])
                    st["wt"] = wt
                    binT = xemb if n == 0 else v[n - 1]
                    cb = kpool.tile([128, H], F16, tag="cb", bufs=(4 if not zero_bias else 5), name="cb")
                    # cb on the (otherwise idle) gpsimd engine
                    nc.gpsimd.tensor_tensor(cb[:], v[n][:], binT[:], ADD)
                    if not zero_bias:
                        cb2 = kpool.tile([128, H], F16, tag="cb2", bufs=3, name="cb2")
                        nc.gpsimd.tensor_tensor(
                            cb2[:], cb[:], bF_sb[:, n * H : (n + 1) * H], ADD
                        )
                        cb = cb2
                    st["cb"] = cb
                    # u1 = tanh(cb)   (inner iter 0; state is zero)
                    u = kpool.tile([128, H], F16, tag="u", bufs=(6 if not zero_bias else 7), name="u")
                    nc.scalar.activation(u[:], cb[:], Tanh, bias=0.0, scale=1.0)
                    st["u"] = u
                else:
                    emit_axpy(st)
                    if j == 4:
                        # v[n] = 0.5 v[n] + 0.25 u5
                        vh = kpool.tile([128, H], F16, tag="vh", bufs=2, name="vh")
                        nc.vector.tensor_scalar_mul(vh[:], v[n][:], 0.5)
                        uq = kpool.tile([128, H], F16, tag="uq", bufs=2, name="uq")
                        nc.vector.tensor_scalar_mul(uq[:], st["u"][:], 0.25)
                        nc.vector.tensor_add(v[n][:], vh[:], uq[:])

            NSTAGE = 5
            T = len(tasks)
            # Per-task start slots.  A task READS v[n] and v[n-1] at its
            # first stage (S0) and WRITES v[n] at its last (S4).  Emission
            # (= program) order must put every read of v[k] after the
            # previous writer's S4, otherwise the reader picks up the stale
            # value.  With the full 10-block schedule consecutive dependent
            # tasks are >= 10 apart and one-task-per-slot is safe; with
            # wavefront skips the sparse head/tail steps bring them as close
            # as 1-2 apart, so stall slots are inserted here.
            start = []
            writer_end = {}
            cur = -1
            for i, (step, n) in enumerate(tasks):
                lo = cur + 1
                for src in ((n, n - 1) if n > 0 else (n,)):
                    if src in writer_end:
                        # same-slot is safe: phase-1 emission is oldest-task
                        # first, so the writer's S4 v-update precedes this
                        # task's S0 read in program order within one slot
                        lo = max(lo, writer_end[src])
                start.append(lo)
                cur = lo
                writer_end[n] = lo + NSTAGE - 1
            total_slots = start[-1] + NSTAGE if T else 0

            states = {}
            nxt = 0
            live = []
            for tau in range(total_slots):
                while nxt < T and start[nxt] == tau:
                    states[nxt] = {"task": tasks[nxt], "ps": psr[nxt % 4]}
                    live.append(nxt)
                    nxt += 1
                for i in list(live):
                    j = tau - start[i]
                    if 0 <= j <= 4:
                        emit_phase1(states[i], j)
                for i in live:
                    j = tau - start[i]
                    if 0 <= j < 4:
                        emit_mms(states[i])
                for i in list(live):
                    j = tau - start[i]
                    if 0 <= j < 4:
                        emit_s_t(states[i])
                    elif j == NSTAGE - 1:
                        del states[i]
                        live.remove(i)

            # ---- head: out^T = head_W @ v[9]^T + head_b  (fp16 matmuls) ----
            # head constants are loaded late so the main-loop weight stream
            # gets the DMA engines first at kernel start
            nc.gpsimd.dma_start(headWT_sb[:], headWT[:])
            nc.gpsimd.dma_start(headB_sb[:], headB[:])
            outsb = kpool.tile([128, KO * BL], F32, tag="outsb", bufs=1)
            ph = psr[1][0]
            for m in range(KO):
                mc = slice(m * 128, (m + 1) * 128)
                for k in range(KH):
                    nc.tensor.matmul(
                        ph[:, mc],
                        headWT_sb[:, k * DOUT + m * 128 : k * DOUT + (m + 1) * 128],
                        v[NB - 1][:, k * 128 : (k + 1) * 128],
                        start=(k == 0),
                        stop=(k == KH - 1),
                    )
                nc.scalar.activation(
                    outsb[:, m * BL : (m + 1) * BL], ph[:, mc], Ident,
                    bias=headB_sb[:, m : m + 1], scale=1.0,
                )
            nc.sync.dma_start(outT[:], outsb[:])
    nc.compile()
    return nc


def _tile_k(a):
    """[K, M] -> [128, (K//128)*M] laid out (k_lo, (k_hi, m))."""
    K, M = a.shape
    return np.ascontiguousarray(
        a.reshape(K // 128, 128, M).transpose(1, 0, 2).reshape(128, (K // 128) * M)
    )


def kernel(**inputs) -> np.ndarray:
    x = np.asarray(inputs["x"], np.float32)
    embed_W = np.asarray(inputs["embed_W"], np.float32)
    embed_b = np.asarray(inputs["embed_b"], np.float32)
    block_W = np.asarray(inputs["block_W"], np.float32)
    block_b = np.asarray(inputs["block_b"], np.float32)
    head_W = np.asarray(inputs["head_W"], np.float32)
    head_b = np.asarray(inputs["head_b"], np.float32)
    steps = int(np.asarray(inputs["steps"]))

    zero_bias = bool(np.all(block_b == 0.0))

    embT2 = embed_W.T.astype(np.float32) * np.float32(0.5)
    Ea = embT2.astype(np.float16)
    Eb = (2.0 * embT2 - Ea.astype(np.float32)).astype(np.float16)
    embWT = np.concatenate([_tile_k(Ea), _tile_k(Eb)], axis=1)
    headWT = _tile_k(head_W.T.astype(np.float16))
    Wt = block_W.transpose(0, 2, 1) * np.float32(0.5)  # [NB, K=h_in, M=d_out]
    Wa = Wt.astype(np.float16)
    Wb = (2.0 * Wt - Wa.astype(np.float32)).astype(np.float16)
    Wab = np.stack(
        [
            np.stack([_tile_k(Wa[n]) for n in range(NB)]),
            np.stack([_tile_k(Wb[n]) for n in range(NB)]),
        ]
    )  # [2, NB, 128, 8*1024] f16
    embB = np.ascontiguousarray(embed_b.reshape(KH, 128).T)
    # bF[p, n*H + m*128 + col] = block_b[n, m*128 + p]  (broadcast along batch)
    bF = np.ascontiguousarray(
        np.broadcast_to(
            block_b.reshape(NB, KH, 128, 1).transpose(2, 0, 1, 3), (128, NB, KH, BL)
        ).reshape(128, NB * H)
    ).astype(np.float16)
    headB = np.ascontiguousarray(head_b.reshape(KO, 128).T)

    in_maps = []
    for ci in range(NCORES):
        xTc = _tile_k(np.ascontiguousarray(x[ci * BL : (ci + 1) * BL].T)).astype(np.float16)
        in_maps.append(
            dict(xT=xTc, embWT=embWT, embB=embB, Wab=Wab, bF=bF,
                 headWT=headWT, headB=headB)
        )

    nc = build_nc(steps, zero_bias)
    res = run_bass_kernel_spmd(nc, in_maps, core_ids=list(range(NCORES)))

    out = np.empty((B, DOUT), np.float32)
    for ci in range(NCORES):
        oT = res.results[ci]["outT"]  # [128, (do_hi=4, b=128)] = out^T tiled
        out[ci * BL : (ci + 1) * BL] = (
            oT.reshape(128, KO, BL).transpose(2, 1, 0).reshape(BL, DOUT)
        )
    return out

